# revision 14
# baseline (speedup 1.0000x reference)
"""Trainium2 Bass kernel for nn_MultiInfAffine.

Math (reference):
    mu_n = mus / ||mus||_D                          [L=6, D=16, K=64]
    t    = <x, mu_n>                                 per (l, n, k)
    d    = arccos(clip(t))
    cost = 0.5 d^2 + alpha
    mc_l = 0.1 * ln sum_k exp(-cost/0.1)
    F    = recurrence over l:  F = wv_l relu(F) + (1-wv_l) mc_l,  wv = exp(-ws^2)
    out  = 0.1 * ln(1 + exp(-10 F))

Device chain per element (branch-free nested half-angle; avoids arccos):
    v   = 1 + t (+delta)          -- folded into the inner-product matmul via an
                                     appended ones-dimension (contract = 17)
    c2  = sqrt(s5 * v)            -- = cos(d/2)            [ACT Sqrt]
    v2  = c2 + 1                  -- [DVE tensor_scalar 2x mode]
    q   = 1/v2                    -- [DVE custom reciprocal_approx_fast]
    m   = (v2 - 2) * q            -- = -tan^2(d/4)         [DVE scalar_tensor_tensor]
    r   = sqrt(-m)                -- = tan(d/4) in [0,1]   [ACT Sqrt]
    a   = arctan(r)               -- = d/4 in [0, pi/4]    [ACT Arctan, in-domain]
    z   = (4 a)^2 = d^2           -- [ACT Square]
    E   = exp(-5 z - 10 alpha)    -- bias per-partition    [ACT Exp] -> bf16
    S_l = sum_k E                 -- block-ones reduction matmul (PE)
then a small tail (Ln + 6-step recurrence + smooth-min) on transposed data.

Layout: 128 SBUF partitions = 2 layers x 64 components ("plane" g covers layers
2g, 2g+1; 3 planes). Points stream along the free axis. ACT instructions are
emitted phase-major per block so activation-table loads (Sqrt / trig / exp sets)
happen only ~3x per block instead of per tile.
"""

import numpy as np
import ml_dtypes

import concourse.bass as bass
import concourse.bacc as bacc
import concourse.tile as tile
from concourse import mybir
from concourse.bass_utils import run_bass_kernel_spmd

N, D, L, K = 250000, 16, 6, 64
NCORES = 8
NPC = N // NCORES  # 31250 true points per core

# tiling (per core)
SC = 512      # points per subtile (columns; one PSUM bank)
NSUB = 16     # subtiles per block
NBLK = 4      # blocks
NPAD = SC * NSUB * NBLK  # 32768 padded points per core
T = NPAD // 128          # 256 point-columns in the tail layout

DELTA = 3e-7             # ones-row pad so v = 1 + t + DELTA > 0 under fp32 noise
S5 = 0.5 * (1.0 - 6e-7)  # sqrt scale keeping s5*v < 1 strictly

F32 = mybir.dt.float32
BF16 = mybir.dt.bfloat16
AF = mybir.ActivationFunctionType
ALU = mybir.AluOpType


def _build(nblk=NBLK, nsub=NSUB, sc=SC, wv=None):
    """Build the per-core Bass program. wv: np.float32[L] = exp(-ws^2)."""
    assert wv is not None
    npad = nblk * nsub * sc
    t_cols = npad // 128
    nsubs_tot = nblk * nsub
    ntt = sc // 128  # point-columns per subtile

    nc = bacc.Bacc()

    xst = nc.dram_tensor("xst", [D + 1, npad], F32, kind="ExternalInput")
    mu = nc.dram_tensor("mu", [D + 1, 3, 128], F32, kind="ExternalInput")
    al = nc.dram_tensor("al", [128, 3], F32, kind="ExternalInput")
    ow = nc.dram_tensor("ow", [128, 3, 6], BF16, kind="ExternalInput")
    fout = nc.dram_tensor("fout", [npad], F32, kind="ExternalOutput")
    sd = nc.dram_tensor("sd", [6, npad], F32)  # staging for S (l-major)

    # recurrence constants
    A = [float(wv[l]) for l in range(L)]
    B = [float((1.0 - wv[l]) * 0.1) for l in range(L)]

    with tile.TileContext(nc) as tc:
        with (
            tc.tile_pool(name="singles", bufs=1) as singles,
            tc.tile_pool(name="xs", bufs=3) as xpool,
            tc.tile_pool(name="vpsum", bufs=3, space="PSUM") as vpool,
            tc.tile_pool(name="spsum", bufs=2, space="PSUM") as spool,
            tc.tile_pool(name="c2", bufs=3) as c2pool,
            tc.tile_pool(name="q", bufs=3) as qpool,
            tc.tile_pool(name="a", bufs=3) as apool,
            tc.tile_pool(name="e", bufs=3) as epool,
            tc.tile_pool(name="rz", bufs=1) as rzpool,
            tc.tile_pool(name="tail", bufs=1) as tailpool,
        ):
            mu_sb = singles.tile([D + 1, 3, 128], F32)
            nc.sync.dma_start(out=mu_sb[:], in_=mu[:])
            al_sb = singles.tile([128, 3], F32)
            nc.sync.dma_start(out=al_sb[:], in_=al[:])
            ow_sb = singles.tile([128, 3, 6], BF16)
            nc.sync.dma_start(out=ow_sb[:], in_=ow[:])

            for b in range(nblk):
                rz = rzpool.tile([128, 3, nsub * sc], F32, tag="rz")
                c2_tiles = {}
                # ---- phase 1: matmul v, ACT Sqrt(c2), DVE chain -> rz = -tan^2(d/4)
                for s in range(nsub):
                    c0 = (b * nsub + s) * sc
                    xs_t = xpool.tile([D + 1, sc], F32, tag="xs")
                    nc.sync.dma_start(out=xs_t[:], in_=xst[:, c0:c0 + sc])
                    for g in range(3):
                        v_t = vpool.tile([128, sc], F32, tag="v")
                        nc.tensor.matmul(v_t[:], mu_sb[:, g, :], xs_t[:])
                        c2_t = c2pool.tile([128, sc], F32, tag="c2")
                        nc.scalar.activation(c2_t[:], v_t[:], AF.Sqrt, scale=S5)
                        c2_tiles[(s, g)] = c2_t
                    for g in range(3):
                        c2_t = c2_tiles[(s, g)]
                        # v2 = c2 + 1 (in place)
                        nc.vector.tensor_scalar_add(c2_t[:], c2_t[:], 1.0)
                        q_t = qpool.tile([128, sc], F32, tag="q")
                        nc.vector.reciprocal_approx_fast(out=q_t[:], in_=c2_t[:])
                        # rz = (v2 - 2) * q = -tan^2(d/4)
                        nc.vector.scalar_tensor_tensor(
                            out=rz[:, g, s * sc:(s + 1) * sc],
                            in0=c2_t[:],
                            scalar=2.0,
                            in1=q_t[:],
                            op0=ALU.subtract,
                            op1=ALU.mult,
                        )
                # ---- phase 2: ACT Sqrt -> r = tan(d/4) (in place, same sqrt set)
                for s in range(nsub):
                    for g in range(3):
                        sl = rz[:, g, s * sc:(s + 1) * sc]
                        nc.scalar.activation(sl, sl, AF.Sqrt, scale=-1.0)
                # ---- phase 3: ACT Arctan + Square (trig set) -> rz = d^2
                for s in range(nsub):
                    for g in range(3):
                        sl = rz[:, g, s * sc:(s + 1) * sc]
                        a_t = apool.tile([128, sc], F32, tag="a")
                        nc.scalar.activation(a_t[:], sl, AF.Arctan)
                        nc.scalar.activation(sl, a_t[:], AF.Square, scale=4.0)
                # ---- phase 4: ACT Exp (exp set) -> E, reduce matmul, stage S
                for s in range(nsub):
                    s_t = spool.tile([6, sc], F32, tag="s")
                    for g in range(3):
                        sl = rz[:, g, s * sc:(s + 1) * sc]
                        e_t = epool.tile([128, sc], BF16, tag="e")
                        nc.scalar.activation(
                            e_t[:], sl, AF.Exp, scale=-5.0, bias=al_sb[:, g:g + 1]
                        )
                        nc.tensor.matmul(
                            s_t[:], ow_sb[:, g, :], e_t[:],
                            start=(g == 0), stop=(g == 2),
                        )
                    sv_t = qpool.tile([6, sc], F32, tag="sv")
                    nc.vector.tensor_copy(sv_t[:], s_t[:])
                    c0 = (b * nsub + s) * sc
                    nc.sync.dma_start(out=sd[:, c0:c0 + sc], in_=sv_t[:])

            # ---- tail: gather-transpose S, Ln, recurrence, smooth-min, store
            mc = tailpool.tile([128, t_cols, 6], F32)
            for l in range(L):
                nc.sync.dma_start(
                    out=mc[:, :, l],
                    in_=sd[l, :].rearrange("(t p) -> p t", p=128),
                )
            nc.scalar.activation(mc[:], mc[:], AF.Ln)
            for l in range(L):
                nc.vector.tensor_scalar_mul(mc[:, :, l], mc[:, :, l], B[l])
            f_t = tailpool.tile([128, t_cols], F32)
            nc.vector.tensor_copy(f_t[:], mc[:, :, 0])
            for l in range(1, L):
                nc.vector.tensor_scalar_max(f_t[:], f_t[:], 0.0)
                nc.vector.scalar_tensor_tensor(
                    out=f_t[:], in0=f_t[:], scalar=A[l], in1=mc[:, :, l],
                    op0=ALU.mult, op1=ALU.add,
                )
            nc.scalar.activation(f_t[:], f_t[:], AF.Exp, scale=-10.0)
            nc.scalar.activation(f_t[:], f_t[:], AF.Ln, bias=1.0)
            nc.vector.tensor_scalar_mul(f_t[:], f_t[:], 0.1)
            nc.sync.dma_start(
                out=fout[:].rearrange("(t p) -> p t", p=128), in_=f_t[:]
            )

    nc.compile()
    return nc


def _host_prep(xs, mus, alphas, ws, npad_per_core=NPAD, ncores=NCORES):
    """Returns (shared inputs dict, list of per-core xst arrays)."""
    mus = np.asarray(mus, np.float32)
    alphas = np.asarray(alphas, np.float32)
    ws = np.asarray(ws, np.float32)
    xs = np.asarray(xs, np.float32)

    mu_n = mus / np.linalg.norm(mus, axis=1, keepdims=True)  # [L, D, K]
    # mu layout: [17, 3, 128]; column j of plane g is (layer 2g + j//64, k = j%64)
    mu_aug = np.zeros((D + 1, 3, 128), np.float32)
    for g in range(3):
        for half in range(2):
            layer = 2 * g + half
            mu_aug[:D, g, 64 * half:64 * half + 64] = mu_n[layer]
    mu_aug[D, :, :] = 1.0 + DELTA

    al = np.zeros((128, 3), np.float32)
    for g in range(3):
        for half in range(2):
            al[64 * half:64 * half + 64, g] = -10.0 * alphas[2 * g + half]

    ow = np.zeros((128, 3, 6), np.float32)
    for g in range(3):
        for half in range(2):
            ow[64 * half:64 * half + 64, g, 2 * g + half] = 1.0
    ow = ow.astype(ml_dtypes.bfloat16)

    wv = np.exp(-ws.astype(np.float32) ** 2).astype(np.float32)

    n = xs.shape[0]
    per = n // ncores
    xst_list = []
    for c in range(ncores):
        shard = xs[c * per:(c + 1) * per]
        aug = np.ones((shard.shape[0], D + 1), np.float32)
        aug[:, :D] = shard
        pad = np.zeros((npad_per_core, D + 1), np.float32)
        pad[:, D] = 1.0  # pad points: x = 0 -> v = 1 + delta, harmless
        pad[:shard.shape[0]] = aug
        xst_list.append(np.ascontiguousarray(pad.T))  # [17, npad]
    return {"mu": mu_aug, "al": al, "ow": ow}, xst_list, wv


def prepare(xs, mus, alphas, ws):
    """Build the Bass program and per-core input maps."""
    shared, xst_list, wv = _host_prep(xs, mus, alphas, ws)
    nc = _build(wv=wv)
    in_maps = [dict(shared, xst=xst_list[c]) for c in range(NCORES)]
    return nc, in_maps


def kernel(xs, mus, alphas, ws, trace=False, tmpdir=None):
    nc, in_maps = prepare(xs, mus, alphas, ws)
    res = run_bass_kernel_spmd(
        nc, in_maps, core_ids=list(range(NCORES)), trace=trace, tmpdir=tmpdir
    )
    per = N // NCORES
    out = np.concatenate([res.results[c]["fout"][:per] for c in range(NCORES)])
    kernel.last_results = res
    return out.astype(np.float32)


# revision 15
# speedup vs baseline: 1.1537x; 1.1537x over previous
"""Trainium2 Bass kernel for nn_MultiInfAffine.

Math (reference):
    mu_n = mus / ||mus||_D                          [L=6, D=16, K=64]
    t    = <x, mu_n>                                 per (l, n, k)
    d    = arccos(clip(t))
    cost = 0.5 d^2 + alpha
    mc_l = 0.1 * ln sum_k exp(-cost/0.1)
    F    = recurrence over l:  F = wv_l relu(F) + (1-wv_l) mc_l,  wv = exp(-ws^2)
    out  = 0.1 * ln(1 + exp(-10 F))

Device chain per element (branch-free nested half-angle; avoids arccos):
    v   = 1 + t (+delta)          -- folded into the inner-product matmul via an
                                     appended ones-dimension (contract = 17)
    c2  = sqrt(s5 * v)            -- = cos(d/2)            [ACT Sqrt]
    v2  = c2 + 1                  -- [DVE tensor_scalar 2x mode]
    q   = 1/v2                    -- [DVE custom reciprocal_approx_fast]
    m   = (v2 - 2) * q            -- = -tan^2(d/4)         [DVE scalar_tensor_tensor]
    r   = sqrt(-m)                -- = tan(d/4) in [0,1]   [ACT Sqrt]
    a   = arctan(r)               -- = d/4 in [0, pi/4]    [ACT Arctan, in-domain]
    z   = (4 a)^2 = d^2           -- [ACT Square]
    E   = exp(-5 z - 10 alpha)    -- bias per-partition    [ACT Exp] -> bf16
    S_l = sum_k E                 -- block-ones reduction matmul (PE)
then a small tail (Ln + 6-step recurrence + smooth-min) on transposed data.

Layout: 128 SBUF partitions = 2 layers x 64 components ("plane" g covers layers
2g, 2g+1; 3 planes). Points stream along the free axis. ACT instructions are
emitted phase-major per block so activation-table loads (Sqrt / trig / exp sets)
happen only ~3x per block instead of per tile.
"""

import numpy as np
import ml_dtypes

import concourse.bass as bass
import concourse.bacc as bacc
import concourse.tile as tile
from concourse import mybir
from concourse.bass_utils import run_bass_kernel_spmd

N, D, L, K = 250000, 16, 6, 64
NCORES = 8
NPC = N // NCORES  # 31250 true points per core

# tiling (per core)
SC = 512      # points per subtile (columns; one PSUM bank)
NSUB = 16     # subtiles per block
NBLK = 4      # blocks
NPAD = SC * NSUB * NBLK  # 32768 padded points per core
T = NPAD // 128          # 256 point-columns in the tail layout

DELTA = 3e-7             # ones-row pad so v = 1 + t + DELTA > 0 under fp32 noise
S5 = 0.5 * (1.0 - 6e-7)  # sqrt scale keeping s5*v < 1 strictly

F32 = mybir.dt.float32
BF16 = mybir.dt.bfloat16
AF = mybir.ActivationFunctionType
ALU = mybir.AluOpType


def _build(nblk=NBLK, nsub=NSUB, sc=SC, wv=None):
    """Build the per-core Bass program. wv: np.float32[L] = exp(-ws^2)."""
    assert wv is not None
    npad = nblk * nsub * sc
    t_cols = npad // 128
    nsubs_tot = nblk * nsub
    ntt = sc // 128  # point-columns per subtile

    nc = bacc.Bacc()

    xst = nc.dram_tensor("xst", [D + 1, npad], F32, kind="ExternalInput")
    mu = nc.dram_tensor("mu", [D + 1, 3, 128], F32, kind="ExternalInput")
    al = nc.dram_tensor("al", [128, 3], F32, kind="ExternalInput")
    ow = nc.dram_tensor("ow", [128, 3, 6], BF16, kind="ExternalInput")
    fout = nc.dram_tensor("fout", [npad], F32, kind="ExternalOutput")
    sd = nc.dram_tensor("sd", [6, npad], F32)  # staging for S (l-major)

    # recurrence constants
    A = [float(wv[l]) for l in range(L)]
    B = [float((1.0 - wv[l]) * 0.1) for l in range(L)]

    with tile.TileContext(nc) as tc:
        with (
            tc.tile_pool(name="singles", bufs=1) as singles,
            tc.tile_pool(name="xs", bufs=3) as xpool,
            tc.tile_pool(name="vpsum", bufs=3, space="PSUM") as vpool,
            tc.tile_pool(name="spsum", bufs=2, space="PSUM") as spool,
            tc.tile_pool(name="c2", bufs=3) as c2pool,
            tc.tile_pool(name="q", bufs=3) as qpool,
            tc.tile_pool(name="a", bufs=3) as apool,
            tc.tile_pool(name="e", bufs=3) as epool,
            tc.tile_pool(name="rz", bufs=1) as rzpool,
            tc.tile_pool(name="tail", bufs=1) as tailpool,
        ):
            mu_sb = singles.tile([D + 1, 3, 128], F32)
            nc.sync.dma_start(out=mu_sb[:], in_=mu[:])
            al_sb = singles.tile([128, 3], F32)
            nc.sync.dma_start(out=al_sb[:], in_=al[:])
            ow_sb = singles.tile([128, 3, 6], BF16)
            nc.sync.dma_start(out=ow_sb[:], in_=ow[:])

            for b in range(nblk):
                rz = rzpool.tile([128, 3, nsub * sc], F32, tag="rz")
                c2_tiles = {}
                # ---- phase 1: matmul v, ACT Sqrt(c2), DVE chain -> rz = -tan^2(d/4)
                for s in range(nsub):
                    c0 = (b * nsub + s) * sc
                    xs_t = xpool.tile([D + 1, sc], F32, tag="xs")
                    nc.sync.dma_start(out=xs_t[:], in_=xst[:, c0:c0 + sc])
                    for g in range(3):
                        v_t = vpool.tile([128, sc], F32, tag="v")
                        nc.tensor.matmul(v_t[:], mu_sb[:, g, :], xs_t[:])
                        c2_t = c2pool.tile([128, sc], F32, tag="c2")
                        nc.scalar.activation(c2_t[:], v_t[:], AF.Sqrt, scale=S5)
                        c2_tiles[(s, g)] = c2_t
                    for g in range(3):
                        c2_t = c2_tiles[(s, g)]
                        # v2 = c2 + 1 (in place)
                        nc.vector.tensor_scalar_add(c2_t[:], c2_t[:], 1.0)
                        q_t = qpool.tile([128, sc], F32, tag="q")
                        nc.vector.reciprocal_approx_fast(out=q_t[:], in_=c2_t[:])
                        # rz = (v2 - 2) * q = -tan^2(d/4)
                        nc.vector.scalar_tensor_tensor(
                            out=rz[:, g, s * sc:(s + 1) * sc],
                            in0=c2_t[:],
                            scalar=2.0,
                            in1=q_t[:],
                            op0=ALU.subtract,
                            op1=ALU.mult,
                        )
                # ---- phase 2: ACT Sqrt -> r = tan(d/4) (in place, same sqrt set)
                for s in range(nsub):
                    for g in range(3):
                        sl = rz[:, g, s * sc:(s + 1) * sc]
                        nc.scalar.activation(sl, sl, AF.Sqrt, scale=-1.0)
                # ---- phase 3: ACT Arctan + Square (trig set) -> rz = d^2
                for s in range(nsub):
                    for g in range(3):
                        sl = rz[:, g, s * sc:(s + 1) * sc]
                        a_t = apool.tile([128, sc], F32, tag="a")
                        nc.scalar.activation(a_t[:], sl, AF.Arctan)
                        nc.scalar.activation(sl, a_t[:], AF.Square, scale=4.0)
                # ---- phase 4: ACT Exp (exp set) -> E, reduce matmul, stage S
                for s in range(nsub):
                    s_t = spool.tile([6, sc], F32, tag="s")
                    for g in range(3):
                        sl = rz[:, g, s * sc:(s + 1) * sc]
                        e_t = epool.tile([128, sc], BF16, tag="e")
                        nc.scalar.activation(
                            e_t[:], sl, AF.Exp, scale=-5.0, bias=al_sb[:, g:g + 1]
                        )
                        nc.tensor.matmul(
                            s_t[:], ow_sb[:, g, :], e_t[:],
                            start=(g == 0), stop=(g == 2),
                        )
                    sv_t = qpool.tile([6, sc], F32, tag="sv")
                    nc.vector.tensor_copy(sv_t[:], s_t[:])
                    c0 = (b * nsub + s) * sc
                    nc.sync.dma_start(out=sd[:, c0:c0 + sc], in_=sv_t[:])

            # ---- tail: reload S with point j on (p = j//T, t = j%T), Ln,
            # recurrence, smooth-min, store. All DMAs are contiguous tile
            # loads/stores because the (p, t) split of j is partition-major.
            mc = tailpool.tile([128, 6, t_cols], F32)
            for l in range(L):
                nc.sync.dma_start(
                    out=mc[:, l, :],
                    in_=sd[l, :].rearrange("(p t) -> p t", p=128),
                )
            nc.scalar.activation(mc[:], mc[:], AF.Ln)
            for l in range(L):
                nc.vector.tensor_scalar_mul(mc[:, l, :], mc[:, l, :], B[l])
            f_t = tailpool.tile([128, t_cols], F32)
            nc.vector.tensor_copy(f_t[:], mc[:, 0, :])
            for l in range(1, L):
                nc.vector.tensor_scalar_max(f_t[:], f_t[:], 0.0)
                nc.vector.scalar_tensor_tensor(
                    out=f_t[:], in0=f_t[:], scalar=A[l], in1=mc[:, l, :],
                    op0=ALU.mult, op1=ALU.add,
                )
            nc.scalar.activation(f_t[:], f_t[:], AF.Exp, scale=-10.0)
            nc.scalar.activation(f_t[:], f_t[:], AF.Ln, bias=1.0)
            nc.vector.tensor_scalar_mul(f_t[:], f_t[:], 0.1)
            nc.sync.dma_start(
                out=fout[:].rearrange("(p t) -> p t", p=128), in_=f_t[:]
            )

    nc.compile()
    return nc


def _host_prep(xs, mus, alphas, ws, npad_per_core=NPAD, ncores=NCORES):
    """Returns (shared inputs dict, list of per-core xst arrays)."""
    mus = np.asarray(mus, np.float32)
    alphas = np.asarray(alphas, np.float32)
    ws = np.asarray(ws, np.float32)
    xs = np.asarray(xs, np.float32)

    mu_n = mus / np.linalg.norm(mus, axis=1, keepdims=True)  # [L, D, K]
    # mu layout: [17, 3, 128]; column j of plane g is (layer 2g + j//64, k = j%64)
    mu_aug = np.zeros((D + 1, 3, 128), np.float32)
    for g in range(3):
        for half in range(2):
            layer = 2 * g + half
            mu_aug[:D, g, 64 * half:64 * half + 64] = mu_n[layer]
    mu_aug[D, :, :] = 1.0 + DELTA

    al = np.zeros((128, 3), np.float32)
    for g in range(3):
        for half in range(2):
            al[64 * half:64 * half + 64, g] = -10.0 * alphas[2 * g + half]

    ow = np.zeros((128, 3, 6), np.float32)
    for g in range(3):
        for half in range(2):
            ow[64 * half:64 * half + 64, g, 2 * g + half] = 1.0
    ow = ow.astype(ml_dtypes.bfloat16)

    wv = np.exp(-ws.astype(np.float32) ** 2).astype(np.float32)

    n = xs.shape[0]
    per = n // ncores
    xst_list = []
    for c in range(ncores):
        shard = xs[c * per:(c + 1) * per]
        aug = np.ones((shard.shape[0], D + 1), np.float32)
        aug[:, :D] = shard
        pad = np.zeros((npad_per_core, D + 1), np.float32)
        pad[:, D] = 1.0  # pad points: x = 0 -> v = 1 + delta, harmless
        pad[:shard.shape[0]] = aug
        xst_list.append(np.ascontiguousarray(pad.T))  # [17, npad]
    return {"mu": mu_aug, "al": al, "ow": ow}, xst_list, wv


def prepare(xs, mus, alphas, ws):
    """Build the Bass program and per-core input maps."""
    shared, xst_list, wv = _host_prep(xs, mus, alphas, ws)
    nc = _build(wv=wv)
    in_maps = [dict(shared, xst=xst_list[c]) for c in range(NCORES)]
    return nc, in_maps


def kernel(xs, mus, alphas, ws, trace=False, tmpdir=None):
    nc, in_maps = prepare(xs, mus, alphas, ws)
    res = run_bass_kernel_spmd(
        nc, in_maps, core_ids=list(range(NCORES)), trace=trace, tmpdir=tmpdir
    )
    per = N // NCORES
    out = np.concatenate([res.results[c]["fout"][:per] for c in range(NCORES)])
    kernel.last_results = res
    return out.astype(np.float32)


# revision 18
# speedup vs baseline: 99.0478x; 85.8509x over previous
"""Trainium2 Bass kernel for nn_MultiInfAffine.

Math (reference):
    mu_n = mus / ||mus||_D                          [L=6, D=16, K=64]
    t    = <x, mu_n>                                 per (l, n, k)
    d    = arccos(clip(t))
    cost = 0.5 d^2 + alpha
    mc_l = 0.1 * ln sum_k exp(-cost/0.1)
    F    = recurrence over l:  F = wv_l relu(F) + (1-wv_l) mc_l,  wv = exp(-ws^2)
    out  = 0.1 * ln(1 + exp(-10 F))

Device chain per element (branch-free nested half-angle; avoids arccos):
    v   = 1 + t (+delta)          -- folded into the inner-product matmul via an
                                     appended ones-dimension (contract = 17)
    c2  = sqrt(s5 * v)            -- = cos(d/2)            [ACT Sqrt]
    v2  = c2 + 1                  -- [DVE tensor_scalar 2x mode]
    q   = 1/v2                    -- [DVE custom reciprocal_approx_fast]
    m   = (v2 - 2) * q            -- = -tan^2(d/4)         [DVE scalar_tensor_tensor]
    r   = sqrt(-m)                -- = tan(d/4) in [0,1]   [ACT Sqrt]
    a   = arctan(r)               -- = d/4 in [0, pi/4]    [ACT Arctan, in-domain]
    z   = (4 a)^2 = d^2           -- [ACT Square]
    E   = exp(-5 z - 10 alpha)    -- bias per-partition    [ACT Exp] -> bf16
    S_l = sum_k E                 -- block-ones reduction matmul (PE)
then a small tail (Ln + 6-step recurrence + smooth-min) on transposed data.

Layout: 128 SBUF partitions = 2 layers x 64 components ("plane" g covers layers
2g, 2g+1; 3 planes). Points stream along the free axis. ACT instructions are
emitted phase-major per block so activation-table loads (Sqrt / trig / exp sets)
happen only ~3x per block instead of per tile.
"""

import numpy as np
import ml_dtypes

import concourse.bass as bass
import concourse.bacc as bacc
import concourse.tile as tile
from concourse import mybir
from concourse.bass_utils import run_bass_kernel_spmd

N, D, L, K = 250000, 16, 6, 64
NCORES = 8
NPC = N // NCORES  # 31250 true points per core

# tiling (per core)
SC = 512      # points per subtile (columns; one PSUM bank)
NSUB = 16     # subtiles per block
NBLK = 4      # blocks
NPAD = SC * NSUB * NBLK  # 32768 padded points per core
T = NPAD // 128          # 256 point-columns in the tail layout

DELTA = 3e-7             # ones-row pad so v = 1 + t + DELTA > 0 under fp32 noise
S5 = 0.5 * (1.0 - 6e-7)  # sqrt scale keeping s5*v < 1 strictly

F32 = mybir.dt.float32
BF16 = mybir.dt.bfloat16
AF = mybir.ActivationFunctionType
ALU = mybir.AluOpType


def _build(nblk=NBLK, nsub=NSUB, sc=SC, wv=None, repeat=1):
    """Build the per-core Bass program. wv: np.float32[L] = exp(-ws^2).
    repeat > 1 wraps the whole body in a HW loop (for timing; idempotent)."""
    assert wv is not None
    npad = nblk * nsub * sc
    t_cols = npad // 128
    nsubs_tot = nblk * nsub
    ntt = sc // 128  # point-columns per subtile

    nc = bacc.Bacc()

    xst = nc.dram_tensor("xst", [D + 1, npad], F32, kind="ExternalInput")
    mu = nc.dram_tensor("mu", [D + 1, 3, 128], F32, kind="ExternalInput")
    al = nc.dram_tensor("al", [128, 3], F32, kind="ExternalInput")
    ow = nc.dram_tensor("ow", [128, 3, 6], BF16, kind="ExternalInput")
    fout = nc.dram_tensor("fout", [npad], F32, kind="ExternalOutput")
    sd = nc.dram_tensor("sd", [6, npad], F32)  # staging for S (l-major)

    # recurrence constants
    A = [float(wv[l]) for l in range(L)]
    B = [float((1.0 - wv[l]) * 0.1) for l in range(L)]

    with tile.TileContext(nc) as tc:
        with (
            tc.tile_pool(name="singles", bufs=1) as singles,
            tc.tile_pool(name="xs", bufs=3) as xpool,
            tc.tile_pool(name="vpsum", bufs=3, space="PSUM") as vpool,
            tc.tile_pool(name="spsum", bufs=2, space="PSUM") as spool,
            tc.tile_pool(name="c2", bufs=3) as c2pool,
            tc.tile_pool(name="q", bufs=3) as qpool,
            tc.tile_pool(name="a", bufs=3) as apool,
            tc.tile_pool(name="e", bufs=3) as epool,
            tc.tile_pool(name="rz", bufs=1) as rzpool,
            tc.tile_pool(name="tail", bufs=1) as tailpool,
        ):
            mu_sb = singles.tile([D + 1, 3, 128], F32)
            nc.sync.dma_start(out=mu_sb[:], in_=mu[:])
            al_sb = singles.tile([128, 3], F32)
            nc.sync.dma_start(out=al_sb[:], in_=al[:])
            ow_sb = singles.tile([128, 3, 6], BF16)
            nc.sync.dma_start(out=ow_sb[:], in_=ow[:])

            def body():
                _emit_body(nc, tc, nblk, nsub, sc, A, B,
                           xst, sd, fout, mu_sb, al_sb, ow_sb,
                           xpool, vpool, spool, c2pool, qpool, apool, epool,
                           rzpool, tailpool)

            if repeat > 1:
                with tc.For_i(0, repeat, 1):
                    body()
            else:
                body()

    nc.compile()
    return nc


def _emit_body(nc, tc, nblk, nsub, sc, A, B,
               xst, sd, fout, mu_sb, al_sb, ow_sb,
               xpool, vpool, spool, c2pool, qpool, apool, epool,
               rzpool, tailpool):
    npad = nblk * nsub * sc
    t_cols = npad // 128
    if True:
        if True:
            for b in range(nblk):
                rz = rzpool.tile([128, 3, nsub * sc], F32, tag="rz")
                c2_tiles = {}
                # ---- phase 1: matmul v, ACT Sqrt(c2), DVE chain -> rz = -tan^2(d/4)
                for s in range(nsub):
                    c0 = (b * nsub + s) * sc
                    xs_t = xpool.tile([D + 1, sc], F32, tag="xs")
                    nc.sync.dma_start(out=xs_t[:], in_=xst[:, c0:c0 + sc])
                    for g in range(3):
                        v_t = vpool.tile([128, sc], F32, tag="v")
                        nc.tensor.matmul(v_t[:], mu_sb[:, g, :], xs_t[:])
                        c2_t = c2pool.tile([128, sc], F32, tag="c2")
                        nc.scalar.activation(c2_t[:], v_t[:], AF.Sqrt, scale=S5)
                        c2_tiles[(s, g)] = c2_t
                    for g in range(3):
                        c2_t = c2_tiles[(s, g)]
                        # v2 = c2 + 1 (in place)
                        nc.vector.tensor_scalar_add(c2_t[:], c2_t[:], 1.0)
                        q_t = qpool.tile([128, sc], F32, tag="q")
                        nc.vector.reciprocal_approx_fast(out=q_t[:], in_=c2_t[:])
                        # rz = (v2 - 2) * q = -tan^2(d/4)
                        nc.vector.scalar_tensor_tensor(
                            out=rz[:, g, s * sc:(s + 1) * sc],
                            in0=c2_t[:],
                            scalar=2.0,
                            in1=q_t[:],
                            op0=ALU.subtract,
                            op1=ALU.mult,
                        )
                # ---- phase 2: ACT Sqrt -> r = tan(d/4) (in place, same sqrt set)
                for s in range(nsub):
                    for g in range(3):
                        sl = rz[:, g, s * sc:(s + 1) * sc]
                        nc.scalar.activation(sl, sl, AF.Sqrt, scale=-1.0)
                # ---- phase 3: ACT Arctan + Square (trig set) -> rz = d^2
                for s in range(nsub):
                    for g in range(3):
                        sl = rz[:, g, s * sc:(s + 1) * sc]
                        a_t = apool.tile([128, sc], F32, tag="a")
                        nc.scalar.activation(a_t[:], sl, AF.Arctan)
                        nc.scalar.activation(sl, a_t[:], AF.Square, scale=4.0)
                # ---- phase 4: ACT Exp (exp set) -> E, reduce matmul, stage S
                for s in range(nsub):
                    s_t = spool.tile([6, sc], F32, tag="s")
                    for g in range(3):
                        sl = rz[:, g, s * sc:(s + 1) * sc]
                        e_t = epool.tile([128, sc], BF16, tag="e")
                        nc.scalar.activation(
                            e_t[:], sl, AF.Exp, scale=-5.0, bias=al_sb[:, g:g + 1]
                        )
                        nc.tensor.matmul(
                            s_t[:], ow_sb[:, g, :], e_t[:],
                            start=(g == 0), stop=(g == 2),
                        )
                    sv_t = qpool.tile([6, sc], F32, tag="sv")
                    nc.vector.tensor_copy(sv_t[:], s_t[:])
                    c0 = (b * nsub + s) * sc
                    nc.sync.dma_start(out=sd[:, c0:c0 + sc], in_=sv_t[:])

            # ---- tail: reload S with point j on (p = j//T, t = j%T), Ln,
            # recurrence, smooth-min, store. All DMAs are contiguous tile
            # loads/stores because the (p, t) split of j is partition-major.
            mc = tailpool.tile([128, 6, t_cols], F32)
            for l in range(L):
                nc.sync.dma_start(
                    out=mc[:, l, :],
                    in_=sd[l, :].rearrange("(p t) -> p t", p=128),
                )
            nc.scalar.activation(mc[:], mc[:], AF.Ln)
            for l in range(L):
                nc.vector.tensor_scalar_mul(mc[:, l, :], mc[:, l, :], B[l])
            f_t = tailpool.tile([128, t_cols], F32)
            nc.vector.tensor_copy(f_t[:], mc[:, 0, :])
            for l in range(1, L):
                nc.vector.tensor_scalar_max(f_t[:], f_t[:], 0.0)
                nc.vector.scalar_tensor_tensor(
                    out=f_t[:], in0=f_t[:], scalar=A[l], in1=mc[:, l, :],
                    op0=ALU.mult, op1=ALU.add,
                )
            nc.scalar.activation(f_t[:], f_t[:], AF.Exp, scale=-10.0)
            nc.scalar.activation(f_t[:], f_t[:], AF.Ln, bias=1.0)
            nc.vector.tensor_scalar_mul(f_t[:], f_t[:], 0.1)
            nc.sync.dma_start(
                out=fout[:].rearrange("(p t) -> p t", p=128), in_=f_t[:]
            )


def _host_prep(xs, mus, alphas, ws, npad_per_core=NPAD, ncores=NCORES):
    """Returns (shared inputs dict, list of per-core xst arrays)."""
    mus = np.asarray(mus, np.float32)
    alphas = np.asarray(alphas, np.float32)
    ws = np.asarray(ws, np.float32)
    xs = np.asarray(xs, np.float32)

    mu_n = mus / np.linalg.norm(mus, axis=1, keepdims=True)  # [L, D, K]
    # mu layout: [17, 3, 128]; column j of plane g is (layer 2g + j//64, k = j%64)
    mu_aug = np.zeros((D + 1, 3, 128), np.float32)
    for g in range(3):
        for half in range(2):
            layer = 2 * g + half
            mu_aug[:D, g, 64 * half:64 * half + 64] = mu_n[layer]
    mu_aug[D, :, :] = 1.0 + DELTA

    al = np.zeros((128, 3), np.float32)
    for g in range(3):
        for half in range(2):
            al[64 * half:64 * half + 64, g] = -10.0 * alphas[2 * g + half]

    ow = np.zeros((128, 3, 6), np.float32)
    for g in range(3):
        for half in range(2):
            ow[64 * half:64 * half + 64, g, 2 * g + half] = 1.0
    ow = ow.astype(ml_dtypes.bfloat16)

    wv = np.exp(-ws.astype(np.float32) ** 2).astype(np.float32)

    n = xs.shape[0]
    per = n // ncores
    xst_list = []
    for c in range(ncores):
        shard = xs[c * per:(c + 1) * per]
        aug = np.ones((shard.shape[0], D + 1), np.float32)
        aug[:, :D] = shard
        pad = np.zeros((npad_per_core, D + 1), np.float32)
        pad[:, D] = 1.0  # pad points: x = 0 -> v = 1 + delta, harmless
        pad[:shard.shape[0]] = aug
        xst_list.append(np.ascontiguousarray(pad.T))  # [17, npad]
    return {"mu": mu_aug, "al": al, "ow": ow}, xst_list, wv


def prepare(xs, mus, alphas, ws):
    """Build the Bass program and per-core input maps."""
    shared, xst_list, wv = _host_prep(xs, mus, alphas, ws)
    nc = _build(wv=wv)
    in_maps = [dict(shared, xst=xst_list[c]) for c in range(NCORES)]
    return nc, in_maps


def kernel(xs, mus, alphas, ws, trace=False, tmpdir=None):
    nc, in_maps = prepare(xs, mus, alphas, ws)
    res = run_bass_kernel_spmd(
        nc, in_maps, core_ids=list(range(NCORES)), trace=trace, tmpdir=tmpdir
    )
    per = N // NCORES
    out = np.concatenate([res.results[c]["fout"][:per] for c in range(NCORES)])
    kernel.last_results = res
    return out.astype(np.float32)


# revision 25
# speedup vs baseline: 164.5988x; 1.6618x over previous
"""Trainium2 Bass kernel for nn_MultiInfAffine.

Math (reference):
    mu_n = mus / ||mus||_D                          [L=6, D=16, K=64]
    t    = <x, mu_n>                                 per (l, n, k)
    d    = arccos(clip(t))
    cost = 0.5 d^2 + alpha
    mc_l = 0.1 * ln sum_k exp(-cost/0.1)
    F    = recurrence over l:  F = wv_l relu(F) + (1-wv_l) mc_l,  wv = exp(-ws^2)
    out  = 0.1 * ln(1 + exp(-10 F))

Device chain per element (branch-free nested half-angle; avoids arccos):
    v   = 1 + t (+delta)          -- folded into the inner-product matmul via an
                                     appended ones-dimension (contract = 17)
    c2  = sqrt(s5 * v)            -- = cos(d/2)            [ACT Sqrt]
    v2  = c2 + 1                  -- [DVE tensor_scalar 2x mode]
    q   = 1/v2                    -- [DVE custom reciprocal_approx_fast]
    m   = (v2 - 2) * q            -- = -tan^2(d/4)         [DVE scalar_tensor_tensor]
    r   = sqrt(-m)                -- = tan(d/4) in [0,1]   [ACT Sqrt]
    a   = arctan(r)               -- = d/4 in [0, pi/4]    [ACT Arctan, in-domain]
    E   = DErf(4*sqrt(5)*a)       -- = (2/sqrt(pi)) exp(-5 d^2)  [ACT] -> bf16
         (sim fallback: Square then Exp)
    S_l = sum_k w_k E_k           -- reduction matmul, weights carry
                                     e^{-10 alpha} (and sqrt(pi)/2 for DErf)
then a small tail (Ln + 6-step recurrence + smooth-min) on re-tiled data.

Layout: 128 SBUF partitions = 2 layers x 64 components ("plane" g covers layers
2g, 2g+1; 3 planes). Points stream along the free axis. ACT instructions are
chained in emission order (add_dep_helper) so activation-table loads stay at
~3 per block instead of per tile; post-c2 ACT passes batch all 3 planes in one
instruction.
"""

import numpy as np
import ml_dtypes

import concourse.bass as bass
import concourse.bacc as bacc
import concourse.tile as tile
from concourse import mybir
from concourse.bass_utils import run_bass_kernel_spmd
from concourse.tile_rust import add_dep_helper

N, D, L, K = 250000, 16, 6, 64
NCORES = 8
NPC = N // NCORES  # 31250 true points per core

# tiling (per core)
SC = 1024     # points per subtile (columns; matmuls split into 512-col halves)
NSUB = 4      # subtiles per block
NBLK = 8      # blocks
NPAD = SC * NSUB * NBLK  # 32768 padded points per core
T = NPAD // 128          # 256 point-columns in the tail layout

DELTA = 3e-7             # ones-row pad so v = 1 + t + DELTA > 0 under fp32 noise
S5 = 0.5 * (1.0 - 6e-7)  # sqrt scale keeping s5*v < 1 strictly
DERF_SCALE = 4.0 * np.sqrt(5.0)  # DErf(4*sqrt(5)*a) = 2/sqrt(pi) exp(-5 d^2)

F32 = mybir.dt.float32
F32R = mybir.dt.float32r
BF16 = mybir.dt.bfloat16
AF = mybir.ActivationFunctionType
ALU = mybir.AluOpType


class _ActChain:
    """Serialize ACT instructions in emission order so the scheduler cannot
    interleave activation-table sets across phases."""

    def __init__(self):
        self.last = None

    def __call__(self, inst):
        if self.last is not None:
            add_dep_helper(inst.ins, self.last.ins, sync=False,
                           reason="act phase order")
        self.last = inst
        return inst


def _build(nblk=NBLK, nsub=NSUB, sc=SC, wv=None, repeat=1, use_derf=True):
    """Build the per-core Bass program. wv: np.float32[L] = exp(-ws^2).
    repeat > 1 wraps the whole body in a HW loop (for timing; idempotent).
    use_derf=False switches to Square+Exp (CoreSim implements those)."""
    assert wv is not None
    npad = nblk * nsub * sc

    nc = bacc.Bacc()

    xst = nc.dram_tensor("xst", [D + 1, npad], F32R, kind="ExternalInput")
    mu = nc.dram_tensor("mu", [D + 1, 3, 128], F32R, kind="ExternalInput")
    ow = nc.dram_tensor("ow", [128, 3, 6], BF16, kind="ExternalInput")
    fout = nc.dram_tensor("fout", [npad], F32, kind="ExternalOutput")
    sd = nc.dram_tensor("sd", [6, npad], F32)  # staging for S (l-major)

    # recurrence constants
    A = [float(wv[l]) for l in range(L)]
    B = [float((1.0 - wv[l]) * 0.1) for l in range(L)]

    with tile.TileContext(nc) as tc:
        with (
            tc.tile_pool(name="singles", bufs=1) as singles,
            tc.tile_pool(name="xs", bufs=3) as xpool,
            tc.tile_pool(name="vpsum", bufs=3, space="PSUM") as vpool,
            tc.tile_pool(name="spsum", bufs=2, space="PSUM") as spool,
            tc.tile_pool(name="c2", bufs=8) as c2pool,
            tc.tile_pool(name="q", bufs=3) as qpool,
            tc.tile_pool(name="e", bufs=2) as epool,
            tc.tile_pool(name="rz", bufs=2) as rzpool,
            tc.tile_pool(name="tail", bufs=1) as tailpool,
        ):
            mu_sb = singles.tile([D + 1, 3, 128], F32R)
            nc.sync.dma_start(out=mu_sb[:], in_=mu[:])
            ow_sb = singles.tile([128, 3, 6], BF16)
            nc.sync.dma_start(out=ow_sb[:], in_=ow[:])

            args = (nc, tc, nblk, nsub, sc, A, B, use_derf,
                    xst, sd, fout, mu_sb, ow_sb,
                    xpool, vpool, spool, c2pool, qpool, epool,
                    rzpool, tailpool)
            if repeat > 1:
                with tc.For_i(0, repeat, 1):
                    _emit_body(*args)
            else:
                _emit_body(*args)

    nc.compile()
    return nc


def _emit_body(nc, tc, nblk, nsub, sc, A, B, use_derf,
               xst, sd, fout, mu_sb, ow_sb,
               xpool, vpool, spool, c2pool, qpool, epool,
               rzpool, tailpool):
    npad = nblk * nsub * sc
    t_cols = npad // 128
    h = sc // 2  # matmul half-width (one PSUM bank)
    act = _ActChain()

    rz_tiles = {}

    def emit_ph1(b):
        # matmul v, ACT Sqrt(c2) [sqrt set], DVE chain -> rz = -tan^2(d/4)
        rz = rzpool.tile([128, 3, nsub * sc], F32, tag="rz")
        rz_tiles[b] = rz
        for s in range(nsub):
            c0 = (b * nsub + s) * sc
            xs_t = xpool.tile([D + 1, sc], F32R, tag="xs")
            nc.sync.dma_start(out=xs_t[:], in_=xst[:, c0:c0 + sc])
            c2_ts = []
            for g in range(3):
                v_t = vpool.tile([128, sc], F32, tag="v")
                nc.tensor.matmul(v_t[:, 0:h], mu_sb[:, g, :], xs_t[:, 0:h])
                nc.tensor.matmul(v_t[:, h:sc], mu_sb[:, g, :], xs_t[:, h:sc])
                c2_t = c2pool.tile([128, sc], F32, tag="c2")
                act(nc.scalar.activation(c2_t[:], v_t[:], AF.Sqrt, scale=S5))
                c2_ts.append(c2_t)
            for g in range(3):
                c2_t = c2_ts[g]
                nc.vector.tensor_scalar_add(c2_t[:], c2_t[:], 1.0)
                q_t = qpool.tile([128, sc], F32, tag="q")
                nc.vector.reciprocal_approx_fast(out=q_t[:], in_=c2_t[:])
                nc.vector.scalar_tensor_tensor(
                    out=rz[:, g, s * sc:(s + 1) * sc],
                    in0=c2_t[:], scalar=2.0, in1=q_t[:],
                    op0=ALU.subtract, op1=ALU.mult,
                )

    def emit_r(b):
        # ACT Sqrt [sqrt set] -> r = tan(d/4), all 3 planes per instr
        rz = rz_tiles[b]
        for s in range(nsub):
            sl = rz[:, :, s * sc:(s + 1) * sc]
            act(nc.scalar.activation(sl, sl, AF.Sqrt, scale=-1.0))

    def emit_atan(b):
        # ACT Arctan [trig set] in place -> rz = d/4
        rz = rz_tiles[b]
        for s in range(nsub):
            sl = rz[:, :, s * sc:(s + 1) * sc]
            act(nc.scalar.activation(sl, sl, AF.Arctan))

    def emit_efold(b):
        # E (bf16) [erf/exp set], reduce matmul, stage S
        rz = rz_tiles[b]
        if not use_derf:
            for s in range(nsub):
                sl = rz[:, :, s * sc:(s + 1) * sc]
                act(nc.scalar.activation(sl, sl, AF.Square, scale=4.0))
        e_ts = {}
        for s in range(nsub):
            sl = rz[:, :, s * sc:(s + 1) * sc]
            e_t = epool.tile([128, 3, sc], BF16, tag="e")
            if use_derf:
                act(nc.scalar.activation(e_t[:], sl, AF.Derivative_Erf,
                                         scale=DERF_SCALE))
            else:
                act(nc.scalar.activation(e_t[:], sl, AF.Exp, scale=-5.0))
            e_ts[s] = e_t
        for s in range(nsub):
            e_t = e_ts[s]
            sv_t = qpool.tile([6, sc], F32, tag="sv")
            for half in range(2):
                s_t = spool.tile([6, h], F32, tag="s")
                for g in range(3):
                    nc.tensor.matmul(
                        s_t[:], ow_sb[:, g, :],
                        e_t[:, g, half * h:(half + 1) * h],
                        start=(g == 0), stop=(g == 2),
                    )
                nc.vector.tensor_copy(sv_t[:, half * h:(half + 1) * h], s_t[:])
            c0 = (b * nsub + s) * sc
            nc.sync.dma_start(out=sd[:, c0:c0 + sc], in_=sv_t[:])
        del rz_tiles[b]

    # Software-pipelined block schedule. Block b+1's c2 phase (sqrt set) is
    # emitted right after block b's r phase (also sqrt set — no table load),
    # giving the DVE chain a full trig+erf phase of lead time.
    emit_ph1(0)
    for b in range(nblk):
        emit_r(b)
        if b + 1 < nblk:
            emit_ph1(b + 1)
        emit_atan(b)
        emit_efold(b)

    # ---- tail: reload S with point j on (p = j//T, t = j%T), Ln,
    # recurrence, smooth-min, store. All DMAs are contiguous because the
    # (p, t) split of j is partition-major.
    mc = tailpool.tile([128, 6, t_cols], F32)
    for l in range(L):
        nc.sync.dma_start(
            out=mc[:, l, :],
            in_=sd[l, :].rearrange("(p t) -> p t", p=128),
        )
    act(nc.scalar.activation(mc[:], mc[:], AF.Ln))
    for l in range(L):
        nc.vector.tensor_scalar_mul(mc[:, l, :], mc[:, l, :], B[l])
    f_t = tailpool.tile([128, t_cols], F32)
    nc.vector.tensor_copy(f_t[:], mc[:, 0, :])
    for l in range(1, L):
        nc.vector.tensor_scalar_max(f_t[:], f_t[:], 0.0)
        nc.vector.scalar_tensor_tensor(
            out=f_t[:], in0=f_t[:], scalar=A[l], in1=mc[:, l, :],
            op0=ALU.mult, op1=ALU.add,
        )
    act(nc.scalar.activation(f_t[:], f_t[:], AF.Exp, scale=-10.0))
    act(nc.scalar.activation(f_t[:], f_t[:], AF.Ln, bias=1.0))
    nc.vector.tensor_scalar_mul(f_t[:], f_t[:], 0.1)
    nc.sync.dma_start(
        out=fout[:].rearrange("(p t) -> p t", p=128), in_=f_t[:]
    )


def _host_prep(xs, mus, alphas, ws, npad_per_core=NPAD, ncores=NCORES,
               use_derf=True):
    """Returns (shared inputs dict, list of per-core xst arrays, wv)."""
    mus = np.asarray(mus, np.float32)
    alphas = np.asarray(alphas, np.float32)
    ws = np.asarray(ws, np.float32)
    xs = np.asarray(xs, np.float32)

    mu_n = mus / np.linalg.norm(mus, axis=1, keepdims=True)  # [L, D, K]
    # mu layout: [17, 3, 128]; column j of plane g is (layer 2g + j//64, k = j%64)
    mu_aug = np.zeros((D + 1, 3, 128), np.float32)
    for g in range(3):
        for half in range(2):
            layer = 2 * g + half
            mu_aug[:D, g, 64 * half:64 * half + 64] = mu_n[layer]
    mu_aug[D, :, :] = 1.0 + DELTA

    # reduction weights carry e^{-10 alpha} (+ sqrt(pi)/2 for the DErf factor)
    wfac = float(np.sqrt(np.pi) / 2.0) if use_derf else 1.0
    ow = np.zeros((128, 3, 6), np.float32)
    for g in range(3):
        for half in range(2):
            layer = 2 * g + half
            ow[64 * half:64 * half + 64, g, layer] = (
                wfac * np.exp(-10.0 * alphas[layer].astype(np.float64))
            ).astype(np.float32)
    ow = ow.astype(ml_dtypes.bfloat16)

    wv = np.exp(-ws.astype(np.float32) ** 2).astype(np.float32)

    n = xs.shape[0]
    per = n // ncores
    xst_list = []
    for c in range(ncores):
        shard = xs[c * per:(c + 1) * per]
        aug = np.ones((shard.shape[0], D + 1), np.float32)
        aug[:, :D] = shard
        pad = np.zeros((npad_per_core, D + 1), np.float32)
        pad[:, D] = 1.0  # pad points: x = 0 -> v = 1 + delta, harmless
        pad[:shard.shape[0]] = aug
        xst_list.append(np.ascontiguousarray(pad.T))  # [17, npad]
    return {"mu": mu_aug, "ow": ow}, xst_list, wv


def prepare(xs, mus, alphas, ws, repeat=1, use_derf=True):
    """Build the Bass program and per-core input maps."""
    shared, xst_list, wv = _host_prep(xs, mus, alphas, ws, use_derf=use_derf)
    nc = _build(wv=wv, repeat=repeat, use_derf=use_derf)
    in_maps = [dict(shared, xst=xst_list[c]) for c in range(NCORES)]
    return nc, in_maps


def kernel(xs, mus, alphas, ws, trace=False, tmpdir=None):
    nc, in_maps = prepare(xs, mus, alphas, ws)
    res = run_bass_kernel_spmd(
        nc, in_maps, core_ids=list(range(NCORES)), trace=trace, tmpdir=tmpdir
    )
    per = N // NCORES
    out = np.concatenate([res.results[c]["fout"][:per] for c in range(NCORES)])
    kernel.last_results = res
    return out.astype(np.float32)


# revision 30
# speedup vs baseline: 166.3335x; 1.0105x over previous
"""Trainium2 Bass kernel for nn_MultiInfAffine.

Math (reference):
    mu_n = mus / ||mus||_D                          [L=6, D=16, K=64]
    t    = <x, mu_n>                                 per (l, n, k)
    d    = arccos(clip(t))
    cost = 0.5 d^2 + alpha
    mc_l = 0.1 * ln sum_k exp(-cost/0.1)
    F    = recurrence over l:  F = wv_l relu(F) + (1-wv_l) mc_l,  wv = exp(-ws^2)
    out  = 0.1 * ln(1 + exp(-10 F))

Device chain per element (branch-free nested half-angle; avoids arccos):
    v   = 1 + t (+delta)          -- folded into the inner-product matmul via an
                                     appended ones-dimension (contract = 17)
    c2  = sqrt(s5 * v)            -- = cos(d/2)            [ACT Sqrt]
    v2  = c2 + 1                  -- [DVE tensor_scalar 2x mode]
    q   = 1/v2                    -- [DVE custom reciprocal_approx_fast]
    m   = (v2 - 2) * q            -- = -tan^2(d/4)         [DVE scalar_tensor_tensor]
    r   = sqrt(-m)                -- = tan(d/4) in [0,1]   [ACT Sqrt]
    a   = arctan(r)               -- = d/4 in [0, pi/4]    [ACT Arctan, in-domain]
    E   = DErf(4*sqrt(5)*a)       -- = (2/sqrt(pi)) exp(-5 d^2)  [ACT] -> bf16
         (sim fallback: Square then Exp)
    S_l = sum_k w_k E_k           -- reduction matmul, weights carry
                                     e^{-10 alpha} (and sqrt(pi)/2 for DErf)
then a small tail (Ln + 6-step recurrence + smooth-min) on re-tiled data.

Layout: 128 SBUF partitions = 2 layers x 64 components ("plane" g covers layers
2g, 2g+1; 3 planes). Points stream along the free axis. ACT instructions are
chained in emission order (add_dep_helper) so activation-table loads stay at
~3 per block instead of per tile; post-c2 ACT passes batch all 3 planes in one
instruction.
"""

import numpy as np
import ml_dtypes

import concourse.bass as bass
import concourse.bacc as bacc
import concourse.tile as tile
from concourse import mybir
from concourse.bass_utils import run_bass_kernel_spmd
from concourse.tile_rust import add_dep_helper

N, D, L, K = 250000, 16, 6, 64
NCORES = 8
NPC = N // NCORES  # 31250 true points per core

# tiling (per core)
SC = 992      # points per subtile (columns; matmul halves fit one PSUM bank)
NSUB = 4      # subtiles per block
NBLK = 8      # blocks
NPAD = SC * NSUB * NBLK  # 31744 padded points per core
T = NPAD // 128          # 248 point-columns in the tail layout

DELTA = 3e-7             # ones-row pad so v = 1 + t + DELTA > 0 under fp32 noise
S5 = 0.5 * (1.0 - 6e-7)  # sqrt scale keeping s5*v < 1 strictly
DERF_SCALE = 4.0 * np.sqrt(5.0)  # DErf(4*sqrt(5)*a) = 2/sqrt(pi) exp(-5 d^2)

F32 = mybir.dt.float32
F32R = mybir.dt.float32r
BF16 = mybir.dt.bfloat16
AF = mybir.ActivationFunctionType
ALU = mybir.AluOpType


class _ActChain:
    """Serialize ACT instructions in emission order so the scheduler cannot
    interleave activation-table sets across phases."""

    def __init__(self):
        self.last = None

    def __call__(self, inst):
        if self.last is not None:
            add_dep_helper(inst.ins, self.last.ins, sync=False,
                           reason="act phase order")
        self.last = inst
        return inst


def _build(nblk=NBLK, nsub=NSUB, sc=SC, wv=None, repeat=1, use_derf=True):
    """Build the per-core Bass program. wv: np.float32[L] = exp(-ws^2).
    repeat > 1 wraps the whole body in a HW loop (for timing; idempotent).
    use_derf=False switches to Square+Exp (CoreSim implements those)."""
    assert wv is not None
    npad = nblk * nsub * sc

    nc = bacc.Bacc()

    xst = nc.dram_tensor("xst", [D + 1, npad], F32R, kind="ExternalInput")
    mu = nc.dram_tensor("mu", [D + 1, 3, 128], F32R, kind="ExternalInput")
    ow = nc.dram_tensor("ow", [128, 3, 6], BF16, kind="ExternalInput")
    fout = nc.dram_tensor("fout", [npad], F32, kind="ExternalOutput")
    sd = nc.dram_tensor("sd", [6, npad], F32)  # staging for S (l-major)

    # recurrence constants
    A = [float(wv[l]) for l in range(L)]
    B = [float((1.0 - wv[l]) * 0.1) for l in range(L)]

    with tile.TileContext(nc) as tc:
        with (
            tc.tile_pool(name="singles", bufs=1) as singles,
            tc.tile_pool(name="xs", bufs=3) as xpool,
            tc.tile_pool(name="vpsum", bufs=3, space="PSUM") as vpool,
            tc.tile_pool(name="spsum", bufs=2, space="PSUM") as spool,
            tc.tile_pool(name="c2", bufs=8) as c2pool,
            tc.tile_pool(name="q", bufs=3) as qpool,
            tc.tile_pool(name="e", bufs=2) as epool,
            tc.tile_pool(name="rz", bufs=2) as rzpool,
            tc.tile_pool(name="tail", bufs=1) as tailpool,
        ):
            mu_sb = singles.tile([D + 1, 3, 128], F32R)
            nc.sync.dma_start(out=mu_sb[:], in_=mu[:])
            ow_sb = singles.tile([128, 3, 6], BF16)
            nc.sync.dma_start(out=ow_sb[:], in_=ow[:])

            args = (nc, tc, nblk, nsub, sc, A, B, use_derf,
                    xst, sd, fout, mu_sb, ow_sb,
                    xpool, vpool, spool, c2pool, qpool, epool,
                    rzpool, tailpool)
            if repeat > 1:
                with tc.For_i(0, repeat, 1):
                    _emit_body(*args)
            else:
                _emit_body(*args)

    nc.compile()
    return nc


def _emit_body(nc, tc, nblk, nsub, sc, A, B, use_derf,
               xst, sd, fout, mu_sb, ow_sb,
               xpool, vpool, spool, c2pool, qpool, epool,
               rzpool, tailpool):
    npad = nblk * nsub * sc
    t_cols = npad // 128
    h = sc // 2  # matmul half-width (one PSUM bank)
    act = _ActChain()

    rz_tiles = {}

    def emit_ph1(b):
        # matmul v, ACT Sqrt(c2) [sqrt set], DVE chain -> rz = -tan^2(d/4)
        rz = rzpool.tile([128, 3, nsub * sc], F32, tag="rz")
        rz_tiles[b] = rz
        for s in range(nsub):
            c0 = (b * nsub + s) * sc
            xs_t = xpool.tile([D + 1, sc], F32R, tag="xs")
            nc.sync.dma_start(out=xs_t[:], in_=xst[:, c0:c0 + sc])
            c2_ts = []
            for g in range(3):
                v_t = vpool.tile([128, sc], F32, tag="v")
                nc.tensor.matmul(v_t[:, 0:h], mu_sb[:, g, :], xs_t[:, 0:h])
                nc.tensor.matmul(v_t[:, h:sc], mu_sb[:, g, :], xs_t[:, h:sc])
                c2_t = c2pool.tile([128, sc], F32, tag="c2")
                act(nc.scalar.activation(c2_t[:], v_t[:], AF.Sqrt, scale=S5))
                c2_ts.append(c2_t)
            for g in range(3):
                c2_t = c2_ts[g]
                nc.vector.tensor_scalar_add(c2_t[:], c2_t[:], 1.0)
                q_t = qpool.tile([128, sc], F32, tag="q")
                nc.vector.reciprocal_approx_fast(out=q_t[:], in_=c2_t[:])
                nc.vector.scalar_tensor_tensor(
                    out=rz[:, g, s * sc:(s + 1) * sc],
                    in0=c2_t[:], scalar=2.0, in1=q_t[:],
                    op0=ALU.subtract, op1=ALU.mult,
                )

    def emit_r(b):
        # ACT Sqrt [sqrt set] -> r = tan(d/4); 3 planes x 2 subtiles per instr
        rz = rz_tiles[b]
        for s in range(0, nsub, 2):
            sl = rz[:, :, s * sc:(s + 2) * sc]
            act(nc.scalar.activation(sl, sl, AF.Sqrt, scale=-1.0))

    def emit_atan(b):
        # ACT Arctan [trig set] in place -> rz = d/4
        rz = rz_tiles[b]
        for s in range(0, nsub, 2):
            sl = rz[:, :, s * sc:(s + 2) * sc]
            act(nc.scalar.activation(sl, sl, AF.Arctan))

    def emit_efold(b):
        # E (bf16) [erf/exp set], reduce matmul, stage S
        rz = rz_tiles[b]
        if not use_derf:
            for s in range(0, nsub, 2):
                sl = rz[:, :, s * sc:(s + 2) * sc]
                act(nc.scalar.activation(sl, sl, AF.Square, scale=4.0))
        e_ts = {}
        for s in range(0, nsub, 2):
            sl = rz[:, :, s * sc:(s + 2) * sc]
            e_t = epool.tile([128, 3, 2 * sc], BF16, tag="e")
            if use_derf:
                act(nc.scalar.activation(e_t[:], sl, AF.Derivative_Erf,
                                         scale=DERF_SCALE))
            else:
                act(nc.scalar.activation(e_t[:], sl, AF.Exp, scale=-5.0))
            e_ts[s] = e_t
            e_ts[s + 1] = None
        for s in range(0, nsub, 2):
            e_t = e_ts[s]
            sv_t = qpool.tile([6, 2 * sc], F32, tag="sv")
            for quarter in range(4):
                s_t = spool.tile([6, h], F32, tag="s")
                for g in range(3):
                    nc.tensor.matmul(
                        s_t[:], ow_sb[:, g, :],
                        e_t[:, g, quarter * h:(quarter + 1) * h],
                        start=(g == 0), stop=(g == 2),
                    )
                nc.vector.tensor_copy(
                    sv_t[:, quarter * h:(quarter + 1) * h], s_t[:])
            c0 = (b * nsub + s) * sc
            nc.sync.dma_start(out=sd[:, c0:c0 + 2 * sc], in_=sv_t[:])
        del rz_tiles[b]

    # Tail buffer: point j lands on (p = j//T, t = j%T); block b covers
    # exactly partitions [pb*b, pb*(b+1)) since nsub*sc is a multiple of T.
    mc = tailpool.tile([128, 6, t_cols], F32)
    blk_cols = nsub * sc
    pb = blk_cols // t_cols
    assert pb * t_cols == blk_cols

    def emit_mc_load(b):
        c0 = b * blk_cols
        for l in range(L):
            nc.sync.dma_start(
                out=mc[pb * b:pb * (b + 1), l, :],
                in_=sd[l, c0:c0 + blk_cols].rearrange("(p t) -> p t", p=pb),
            )

    # Software-pipelined block schedule. Block b+1's c2 phase (sqrt set) is
    # emitted right after block b's r phase (also sqrt set — no table load),
    # giving the DVE chain a full trig+erf phase of lead time.
    emit_ph1(0)
    for b in range(nblk):
        emit_r(b)
        if b + 1 < nblk:
            emit_ph1(b + 1)
        emit_atan(b)
        emit_efold(b)
        emit_mc_load(b)

    # ---- tail: Ln, recurrence, smooth-min, store
    act(nc.scalar.activation(mc[:], mc[:], AF.Ln))
    for l in range(L):
        nc.vector.tensor_scalar_mul(mc[:, l, :], mc[:, l, :], B[l])
    f_t = tailpool.tile([128, t_cols], F32)
    nc.vector.tensor_copy(f_t[:], mc[:, 0, :])
    for l in range(1, L):
        nc.vector.tensor_scalar_max(f_t[:], f_t[:], 0.0)
        nc.vector.scalar_tensor_tensor(
            out=f_t[:], in0=f_t[:], scalar=A[l], in1=mc[:, l, :],
            op0=ALU.mult, op1=ALU.add,
        )
    act(nc.scalar.activation(f_t[:], f_t[:], AF.Exp, scale=-10.0))
    act(nc.scalar.activation(f_t[:], f_t[:], AF.Ln, bias=1.0))
    nc.vector.tensor_scalar_mul(f_t[:], f_t[:], 0.1)
    nc.sync.dma_start(
        out=fout[:].rearrange("(p t) -> p t", p=128), in_=f_t[:]
    )


def _host_prep(xs, mus, alphas, ws, npad_per_core=NPAD, ncores=NCORES,
               use_derf=True):
    """Returns (shared inputs dict, list of per-core xst arrays, wv)."""
    mus = np.asarray(mus, np.float32)
    alphas = np.asarray(alphas, np.float32)
    ws = np.asarray(ws, np.float32)
    xs = np.asarray(xs, np.float32)

    mu_n = mus / np.linalg.norm(mus, axis=1, keepdims=True)  # [L, D, K]
    # mu layout: [17, 3, 128]; column j of plane g is (layer 2g + j//64, k = j%64)
    mu_aug = np.zeros((D + 1, 3, 128), np.float32)
    for g in range(3):
        for half in range(2):
            layer = 2 * g + half
            mu_aug[:D, g, 64 * half:64 * half + 64] = mu_n[layer]
    mu_aug[D, :, :] = 1.0 + DELTA

    # reduction weights carry e^{-10 alpha} (+ sqrt(pi)/2 for the DErf factor)
    wfac = float(np.sqrt(np.pi) / 2.0) if use_derf else 1.0
    ow = np.zeros((128, 3, 6), np.float32)
    for g in range(3):
        for half in range(2):
            layer = 2 * g + half
            ow[64 * half:64 * half + 64, g, layer] = (
                wfac * np.exp(-10.0 * alphas[layer].astype(np.float64))
            ).astype(np.float32)
    ow = ow.astype(ml_dtypes.bfloat16)

    wv = np.exp(-ws.astype(np.float32) ** 2).astype(np.float32)

    n = xs.shape[0]
    per = n // ncores
    xst_list = []
    for c in range(ncores):
        shard = xs[c * per:(c + 1) * per]
        aug = np.ones((shard.shape[0], D + 1), np.float32)
        aug[:, :D] = shard
        pad = np.zeros((npad_per_core, D + 1), np.float32)
        pad[:, D] = 1.0  # pad points: x = 0 -> v = 1 + delta, harmless
        pad[:shard.shape[0]] = aug
        xst_list.append(np.ascontiguousarray(pad.T))  # [17, npad]
    return {"mu": mu_aug, "ow": ow}, xst_list, wv


def prepare(xs, mus, alphas, ws, repeat=1, use_derf=True):
    """Build the Bass program and per-core input maps."""
    shared, xst_list, wv = _host_prep(xs, mus, alphas, ws, use_derf=use_derf)
    nc = _build(wv=wv, repeat=repeat, use_derf=use_derf)
    in_maps = [dict(shared, xst=xst_list[c]) for c in range(NCORES)]
    return nc, in_maps


def kernel(xs, mus, alphas, ws, trace=False, tmpdir=None):
    nc, in_maps = prepare(xs, mus, alphas, ws)
    res = run_bass_kernel_spmd(
        nc, in_maps, core_ids=list(range(NCORES)), trace=trace, tmpdir=tmpdir
    )
    per = N // NCORES
    out = np.concatenate([res.results[c]["fout"][:per] for c in range(NCORES)])
    kernel.last_results = res
    return out.astype(np.float32)
